# revision 27
# baseline (speedup 1.0000x reference)
"""nn_GT_7327214207519 — 2-layer TransformerConv GNN (heads=4) on 8 trn2 NeuronCores.

Design notes (this runtime executes roughly one instruction per ~70us, so the
program is shaped to minimize INSTRUCTION COUNT above all):
  * Nodes are split into 8 contiguous ranges (2500/core); each core owns the
    destination-side softmax + aggregation for its range (no cross-core
    reduction for attention).
  * Edges are processed in flat chunks of 1024: one dma_gather for [K|V] rows,
    one for q rows, a handful of batched strided/broadcast DVE ops for the
    logits/exp/weighted messages, then ONE dma_scatter_add that accumulates
    [alpha*v | alpha] rows per destination in an HBM table. A dense pass then
    normalizes per destination.
  * Softmax max-subtraction is skipped (logits are O(1) here); biases are
    folded (bq into q table, bk cancels in softmax, bv/bs into dense adds).
  * k/v tables are computed sharded (own rows only) and AllGathered on-device;
    x and the weights also arrive sharded and are AllGathered — the
    host<->device tunnel is slow, so each core receives ONE ~2.3MB blob.
"""

import math
import os
import numpy as np
import ml_dtypes



BF = ml_dtypes.bfloat16

# Problem constants (fixed by the task; kernel.py must be self-contained).
N_NODES, N_EDGES, D_IN, HID, OUT_CH, H = 20000, 320000, 128, 128, 128, 4
C = 128            # per-head channels, both layers
D = H * C          # 512
RANKS = 8
TW = 576           # scatter-table row: [alpha*v (512) | alpha (4) | pad] f32; 2304B % 256 == 0

FULL_CFG = dict(N=N_NODES, RANKS=RANKS, NB=20, T=17)

# weights blob: w0kv [128,2D] | w0q [128,D] | w0s [128,D] |
#               w1kv [128,4,2D] | w1q [128,4,D] | w1s [128,4,OUT_CH]   (bf16)
W_SIZES = [128 * 2 * D, 128 * D, 128 * D, 128 * 4 * 2 * D, 128 * 4 * D,
           128 * 4 * OUT_CH]
WTOT = sum(W_SIZES) * 2            # bytes
W_OFFS = np.cumsum([0] + W_SIZES)[:-1] * 2


def derive(cfg):
    g = dict(cfg)
    g["PER"] = g["N"] // g["RANKS"]          # real nodes per rank
    g["PERP"] = g["NB"] * 128                # padded nodes per rank
    assert g["PERP"] >= g["PER"]
    g["NTAB"] = g["RANKS"] * g["PERP"]       # padded kv-table rows
    g["NI"] = g["T"] * 128                   # edge slots per dst block
    assert g["NTAB"] < 32768                 # int16 gather indices
    wfull_b = WTOT + (3 * D + OUT_CH) * 4    # weights + bias rows
    wfull_b += -wfull_b % g["RANKS"]
    g["WSH"] = wfull_b // g["RANKS"]         # weight shard bytes per rank
    ic = 128 * g["NB"] * (g["NI"] // 16) * 2
    sizes = dict(
        x_own=g["PERP"] * D_IN * 2,
        wsh=g["WSH"],
        kvidx=ic,                            # wrapped int16, [128, NB*NI/16]
        dqidx=ic,                            # local dst idx for the q gather
        dstrel=128 * g["NB"] * g["T"] * 2,   # bf16 dst-within-block, [128, NB*T]
        iota=128 * 128 * 2,
    )
    offs, off = {}, 0
    for k, s in sizes.items():
        offs[k] = off
        off += s
    g["BLOB_OFFS"], g["BLOB_BYTES"] = offs, off
    return g


# ----------------------------------------------------------------------------
# Program builder
# ----------------------------------------------------------------------------

def build_program(cfg):
    import concourse.bass as bass
    import concourse.mybir as mybir
    import concourse.tile as tile
    from concourse import bacc
    from concourse.masks import make_identity

    g = derive(cfg)
    NB, T, NI, NTAB, PERP, WSH = (g["NB"], g["T"], g["NI"], g["NTAB"], g["PERP"],
                                  g["WSH"])
    NRANKS = g["RANKS"]
    OFFS = g["BLOB_OFFS"]
    F32, BF16, I16, U8 = (mybir.dt.float32, mybir.dt.bfloat16, mybir.dt.int16,
                          mybir.dt.uint8)
    AF = mybir.ActivationFunctionType
    OP = mybir.AluOpType
    SCALE = 1.0 / math.sqrt(C)

    nc = bacc.Bacc("TRN2", target_bir_lowering=False, debug=False,
                   num_devices=NRANKS)

    blob = nc.dram_tensor("blob", [g["BLOB_BYTES"]], U8, kind="ExternalInput").ap()
    out_t = nc.dram_tensor("out", [PERP, OUT_CH], BF16, kind="ExternalOutput").ap()

    def bv(key, nbytes):
        o = OFFS[key]
        return blob[o:o + nbytes]

    rg = [list(range(NRANKS))]

    with tile.TileContext(nc) as tc:
        with (
            tc.tile_pool(name="dram", bufs=1, space="DRAM") as dpool,
            tc.tile_pool(name="const", bufs=1) as cpool,
        ):
            kv0_own = dpool.tile([PERP, 2 * D], BF16, tag="kv0o")
            kv0_t = dpool.tile([NTAB, 2 * D], BF16, tag="kv0", addr_space="Shared")
            kv1_own = dpool.tile([PERP, 2 * D], BF16, tag="kv1o")
            kv1_t = dpool.tile([NTAB, 2 * D], BF16, tag="kv1", addr_space="Shared")
            q0_t = dpool.tile([PERP + 128, D], BF16, tag="q0")
            q1_t = dpool.tile([PERP + 128, D], BF16, tag="q1")
            sk0_t = dpool.tile([PERP, D], F32, tag="sk0")
            sk1_t = dpool.tile([PERP, OUT_CH], F32, tag="sk1")
            wb = dpool.tile([WSH], U8, tag="wb")
            wfull = dpool.tile([NRANKS * WSH], U8, tag="wfull", addr_space="Shared")

            nc.sync.dma_start(out=wb[:], in_=bv("wsh", WSH))
            nc.gpsimd.collective_compute(
                "AllGather", OP.bypass, replica_groups=rg,
                ins=[wb.opt()], outs=[wfull.opt()])

            def wview(i, shape_str, **kw):
                v = wfull[int(W_OFFS[i]):int(W_OFFS[i]) + W_SIZES[i] * 2]
                return v.bitcast(BF16).rearrange(shape_str, **kw)

            w0kv_s = cpool.tile([128, 2 * D], BF16, tag="w0kv")
            nc.sync.dma_start(out=w0kv_s[:], in_=wview(0, "(p d) -> p d", p=128))
            w0q_s = cpool.tile([128, D], BF16, tag="w0q")
            nc.sync.dma_start(out=w0q_s[:], in_=wview(1, "(p d) -> p d", p=128))
            w0s_s = cpool.tile([128, D], BF16, tag="w0s")
            nc.sync.dma_start(out=w0s_s[:], in_=wview(2, "(p d) -> p d", p=128))
            w1kv_s = cpool.tile([128, 4, 2 * D], BF16, tag="w1kv")
            nc.sync.dma_start(out=w1kv_s[:], in_=wview(3, "(p s d) -> p s d", p=128, s=4))
            w1q_s = cpool.tile([128, 4, D], BF16, tag="w1q")
            nc.sync.dma_start(out=w1q_s[:], in_=wview(4, "(p s d) -> p s d", p=128, s=4))
            w1s_s = cpool.tile([128, 4, OUT_CH], BF16, tag="w1s")
            nc.sync.dma_start(out=w1s_s[:], in_=wview(5, "(p s d) -> p s d", p=128, s=4))

            ident_s = cpool.tile([128, 128], BF16, tag="ident")
            make_identity(nc, ident_s[:])
            iota_s = cpool.tile([128, 128], BF16, tag="iota")
            nc.sync.dma_start(out=iota_s[:],
                              in_=bv("iota", 128 * 128 * 2).bitcast(BF16)
                              .rearrange("(p d) -> p d", p=128))

            # bias rows live in the last 6656 bytes of the weights blob (see host_prep)
            ones_s = cpool.tile([1, 128], F32, tag="ones")
            nc.vector.memset(ones_s[:], 1.0)
            brow_s = cpool.tile([1, 3 * D + OUT_CH], F32, tag="brow")
            boff = int(W_OFFS[5]) + W_SIZES[5] * 2
            nc.sync.dma_start(out=brow_s[:],
                              in_=wfull[boff:boff + (3 * D + OUT_CH) * 4]
                              .bitcast(F32).rearrange("(o d) -> o d", o=1))
            b0q_s = cpool.tile([128, D], F32, tag="b0q")
            c0_s = cpool.tile([128, D], F32, tag="c0")
            b1q_s = cpool.tile([128, D], F32, tag="b1q")
            c1_s = cpool.tile([128, OUT_CH], F32, tag="c1")

            with (
                tc.tile_pool(name="work", bufs=1) as pool,
                tc.tile_pool(name="roll", bufs=3) as rpool,
                tc.tile_pool(name="psum", bufs=1, space="PSUM") as pp,
            ):
                for bi, (btile, w) in enumerate(
                        [(b0q_s, D), (c0_s, D), (b1q_s, D), (c1_s, OUT_CH)]):
                    pb = pp.tile([128, D], F32, tag="pa")
                    nc.tensor.matmul(pb[:, :w], ones_s[:],
                                     brow_s[:, bi * D:bi * D + w], start=True, stop=True)
                    nc.scalar.activation(btile[:], pb[:, :w], AF.Copy)

                # ---------------- P0: layer-0 projections (own rows only) -------
                x_own_v = bv("x_own", PERP * D_IN * 2).bitcast(BF16).rearrange(
                    "(n d) -> n d", d=D_IN)
                for btg in range(NB // 4):
                    xtg = rpool.tile([128, 512], BF16, tag="xtg")
                    nc.sync.dma_start(out=xtg[:],
                                      in_=x_own_v[btg * 512:(btg + 1) * 512, :],
                                      transpose=True)
                    for sub in range(4):
                        bt = btg * 4 + sub
                        lhsT = xtg[:, sub * 128:(sub + 1) * 128]
                        pk = pp.tile([128, D], F32, tag="pk")
                        pv = pp.tile([128, D], F32, tag="pv")
                        nc.tensor.matmul(pk[:], lhsT, w0kv_s[:, 0:D], start=True, stop=True)
                        nc.tensor.matmul(pv[:], lhsT, w0kv_s[:, D:2 * D], start=True, stop=True)
                        kvb = rpool.tile([128, 2 * D], BF16, tag="kvb")
                        nc.scalar.activation(kvb[:, 0:D], pk[:], AF.Copy)
                        nc.vector.tensor_copy(out=kvb[:, D:2 * D], in_=pv[:])
                        nc.sync.dma_start(out=kv0_own[bt * 128:(bt + 1) * 128, :], in_=kvb[:])
                        pq = pp.tile([128, D], F32, tag="pk")
                        nc.tensor.matmul(pq[:], lhsT, w0q_s[:], start=True, stop=True)
                        qb = rpool.tile([128, D], BF16, tag="kvb")
                        nc.vector.tensor_tensor(out=qb[:], in0=pq[:], in1=b0q_s[:], op=OP.add)
                        nc.sync.dma_start(out=q0_t[bt * 128:(bt + 1) * 128, :], in_=qb[:])
                        ps = pp.tile([128, D], F32, tag="pv")
                        nc.tensor.matmul(ps[:], lhsT, w0s_s[:], start=True, stop=True)
                        skb = rpool.tile([128, D], F32, tag="skb")
                        nc.vector.tensor_tensor(out=skb[:], in0=ps[:], in1=c0_s[:], op=OP.add)
                        nc.sync.dma_start(out=sk0_t[bt * 128:(bt + 1) * 128, :], in_=skb[:])
                nc.gpsimd.collective_compute(
                    "AllGather", OP.bypass, replica_groups=rg,
                    ins=[kv0_own.opt()], outs=[kv0_t.opt()])

                kvidx_v = bv("kvidx", 128 * NB * (NI // 16) * 2).bitcast(I16).rearrange(
                    "(p s) -> p s", p=128)
                dqidx_v = bv("dqidx", 128 * NB * (NI // 16) * 2).bitcast(I16).rearrange(
                    "(p s) -> p s", p=128)
                dstrel_v = bv("dstrel", 128 * NB * T * 2).bitcast(BF16).rearrange(
                    "(p s) -> p s", p=128)

                # ---------------- edge phase (both layers) ----------------
                def edge_layer(layer):
                    kv_tab = kv0_t if layer == 0 else kv1_t
                    q_tab = q0_t if layer == 0 else q1_t
                    ikv_all = cpool.tile([128, NB * (NI // 16)], I16, tag=f"ikv{layer}")
                    nc.sync.dma_start(out=ikv_all[:], in_=kvidx_v)
                    idq_all = cpool.tile([128, NB * (NI // 16)], I16, tag=f"idq{layer}")
                    nc.sync.dma_start(out=idq_all[:], in_=dqidx_v)
                    dr_all = cpool.tile([128, NB * T], BF16, tag=f"dr{layer}")
                    nc.sync.dma_start(out=dr_all[:], in_=dstrel_v)
                    for b in range(NB):
                        i0 = b * (NI // 16)
                        kvt = pool.tile([128, T, 2 * D], BF16, tag="kvt")
                        qt = pool.tile([128, T, D], BF16, tag="qt")
                        for g0 in range(0, T, 8):
                            gt = min(8, T - g0)
                            ni = gt * 128
                            nc.gpsimd.dma_gather(
                                kvt[:, g0:g0 + gt, :], kv_tab[:],
                                ikv_all[:, i0 + g0 * 8:i0 + g0 * 8 + ni // 16],
                                ni, ni, elem_size=2 * D)
                            nc.gpsimd.dma_gather(
                                qt[:, g0:g0 + gt, :], q_tab[:],
                                idq_all[:, i0 + g0 * 8:i0 + g0 * 8 + ni // 16],
                                ni, ni, elem_size=D)
                        qk = pool.tile([128, T, D], BF16, tag="qk")
                        nc.vector.tensor_tensor(out=qk[:], in0=qt[:],
                                                in1=kvt[:, :, 0:D], op=OP.mult)
                        alpha = rpool.tile([128, 4 * T], F32, tag="alpha")
                        nc.vector.tensor_reduce(
                            out=alpha[:].rearrange("p (t h) -> p t h", h=H),
                            in_=qk[:].rearrange("p t (h c) -> p t h c", c=C),
                            axis=mybir.AxisListType.X, op=OP.add)
                        ae = rpool.tile([128, 4 * T], F32, tag="ae")
                        nc.scalar.activation(ae[:], alpha[:], AF.Exp, scale=SCALE)
                        aeb = rpool.tile([128, 4 * T], BF16, tag="aeb")
                        nc.vector.tensor_copy(out=aeb[:], in_=ae[:])
                        ct = pool.tile([128, T, D], BF16, tag="ct")
                        nc.vector.tensor_tensor(
                            out=ct[:].rearrange("p t (h c) -> p t h c", c=C),
                            in0=kvt[:, :, D:2 * D].rearrange("p t (h c) -> p t h c", c=C),
                            in1=aeb[:].rearrange("p (t h o) -> p t h o", h=H, o=1)
                            .to_broadcast([128, T, H, C]),
                            op=OP.mult)
                        Ma = pool.tile([128, T, 128], BF16, tag="Ma")
                        nc.vector.tensor_tensor(
                            out=Ma[:],
                            in0=dr_all[:, b * T:(b + 1) * T]
                            .rearrange("p (t o) -> p t o", o=1)
                            .to_broadcast([128, T, 128]),
                            in1=iota_s[:].rearrange("p (o d) -> p o d", o=1)
                            .to_broadcast([128, T, 128]),
                            op=OP.is_equal)
                        po = pp.tile([128, D], F32, tag="po")
                        ps = pp.tile([128, 4], F32, tag="ps")
                        for j in range(T):
                            Mj = Ma[:, j:j + 1, :].rearrange("p o d -> p (o d)")
                            nc.tensor.matmul(ps[:], Mj, aeb[:, 4 * j:4 * j + 4],
                                             start=(j == 0), stop=(j == T - 1))
                            nc.tensor.matmul(po[:], Mj,
                                             ct[:, j:j + 1, :].rearrange("p o d -> p (o d)"),
                                             start=(j == 0), stop=(j == T - 1))

                        # ---- block finalize ----
                        rsl = slice(b * 128, (b + 1) * 128)
                        se = rpool.tile([128, H], F32, tag="se")
                        nc.vector.tensor_scalar_add(se[:], ps[:], 1e-30)
                        iv = rpool.tile([128, H], F32, tag="iv")
                        nc.vector.reciprocal(out=iv[:], in_=se[:])
                        if layer == 0:
                            hb = rpool.tile([128, D], F32, tag="hb")
                            nc.vector.tensor_tensor(
                                out=hb[:].rearrange("p (h c) -> p h c", c=C),
                                in0=po[:].rearrange("p (h c) -> p h c", c=C),
                                in1=iv[:].to_broadcast([128, H, C]),
                                op=OP.mult)
                            skb = rpool.tile([128, D], F32, tag="skb")
                            nc.sync.dma_start(out=skb[:], in_=sk0_t[rsl, :])
                            nc.vector.tensor_tensor(out=hb[:], in0=hb[:], in1=skb[:], op=OP.add)
                            hbb = rpool.tile([128, D], BF16, tag="hbb")
                            nc.scalar.activation(hbb[:], hb[:], AF.Relu)
                            hTall = rpool.tile([128, 4, 128], BF16, tag="hTall")
                            for sg in range(4):
                                pt = pp.tile([128, 128], BF16, tag="pa")
                                nc.tensor.transpose(pt[:], hbb[:, sg * 128:(sg + 1) * 128],
                                                    ident_s[:])
                                nc.scalar.activation(
                                    hTall[:, sg:sg + 1, :].rearrange("p o d -> p (o d)"),
                                    pt[:], AF.Copy)
                            pq = pp.tile([128, D], F32, tag="pq")
                            for sg in range(4):
                                nc.tensor.matmul(
                                    pq[:], hTall[:, sg:sg + 1, :].rearrange("p o d -> p (o d)"),
                                    w1q_s[:, sg:sg + 1, :].rearrange("p o d -> p (o d)"),
                                    start=(sg == 0), stop=(sg == 3))
                            qb = rpool.tile([128, D], BF16, tag="qb1")
                            nc.vector.tensor_tensor(out=qb[:], in0=pq[:], in1=b1q_s[:], op=OP.add)
                            nc.sync.dma_start(out=q1_t[rsl, :], in_=qb[:])
                            pkk = pp.tile([128, D], F32, tag="pkk")
                            pvv = pp.tile([128, D], F32, tag="pvv")
                            for sg in range(4):
                                lhsT = hTall[:, sg:sg + 1, :].rearrange("p o d -> p (o d)")
                                nc.tensor.matmul(pkk[:], lhsT,
                                                 w1kv_s[:, sg:sg + 1, 0:D].rearrange("p o d -> p (o d)"),
                                                 start=(sg == 0), stop=(sg == 3))
                                nc.tensor.matmul(pvv[:], lhsT,
                                                 w1kv_s[:, sg:sg + 1, D:2 * D].rearrange("p o d -> p (o d)"),
                                                 start=(sg == 0), stop=(sg == 3))
                            kvb = rpool.tile([128, 2 * D], BF16, tag="kvb1")
                            nc.scalar.activation(kvb[:, 0:D], pkk[:], AF.Copy)
                            nc.scalar.activation(kvb[:, D:2 * D], pvv[:], AF.Copy)
                            nc.sync.dma_start(out=kv1_own[rsl, :], in_=kvb[:])
                            psk = pp.tile([128, OUT_CH], F32, tag="pq")
                            for sg in range(4):
                                nc.tensor.matmul(
                                    psk[:], hTall[:, sg:sg + 1, :].rearrange("p o d -> p (o d)"),
                                    w1s_s[:, sg:sg + 1, :].rearrange("p o d -> p (o d)"),
                                    start=(sg == 0), stop=(sg == 3))
                            s1b = rpool.tile([128, OUT_CH], F32, tag="s1b")
                            nc.vector.tensor_tensor(out=s1b[:], in0=psk[:], in1=c1_s[:], op=OP.add)
                            nc.sync.dma_start(out=sk1_t[rsl, :], in_=s1b[:])
                        else:
                            iv4 = rpool.tile([128, H], F32, tag="iv4")
                            nc.vector.tensor_scalar_mul(iv4[:], iv[:], 1.0 / H)
                            nrm = rpool.tile([128, D], F32, tag="nrm")
                            nc.vector.tensor_tensor(
                                out=nrm[:].rearrange("p (h c) -> p h c", c=C),
                                in0=po[:].rearrange("p (h c) -> p h c", c=C),
                                in1=iv4[:].to_broadcast([128, H, C]),
                                op=OP.mult)
                            mn = rpool.tile([128, OUT_CH], F32, tag="mn")
                            nc.vector.tensor_reduce(
                                out=mn[:], in_=nrm[:].rearrange("p (h c) -> p c h", c=C),
                                axis=mybir.AxisListType.X, op=OP.add)
                            skb = rpool.tile([128, OUT_CH], F32, tag="skb1")
                            nc.sync.dma_start(out=skb[:], in_=sk1_t[rsl, :])
                            ob = rpool.tile([128, OUT_CH], BF16, tag="ob")
                            nc.vector.tensor_tensor(out=ob[:], in0=mn[:], in1=skb[:], op=OP.add)
                            nc.sync.dma_start(out=out_t[rsl, :], in_=ob[:])
                    if layer == 0:
                        nc.gpsimd.collective_compute(
                            "AllGather", OP.bypass, replica_groups=rg,
                            ins=[kv1_own.opt()], outs=[kv1_t.opt()])

                edge_layer(0)
                edge_layer(1)


    nc.compile()
    return nc


# ----------------------------------------------------------------------------
# Host-side preparation
# ----------------------------------------------------------------------------

def host_prep(cfg, x, edge_index,
              Wq0, bq0, Wk0, bk0, Wv0, bv0, Ws0, bs0,
              Wq1, bq1, Wk1, bk1, Wv1, bv1, Ws1, bs1):
    g = derive(cfg)
    NRANKS, NB, T, NI, PER, PERP = (g["RANKS"], g["NB"], g["T"], g["NI"],
                                    g["PER"], g["PERP"])
    OFFS, TOT = g["BLOB_OFFS"], g["BLOB_BYTES"]

    x = np.asarray(x, np.float32)
    src = np.asarray(edge_index[0], np.int64)
    dst = np.asarray(edge_index[1], np.int64)
    nprime = (src // PER) * PERP + (src % PER)   # src id in padded kv tables

    w0kv = np.concatenate([np.asarray(Wk0, np.float32).astype(BF),
                           np.asarray(Wv0, np.float32).astype(BF)], 1)
    bias = np.concatenate([
        np.asarray(bq0, np.float32),
        (np.asarray(bs0) + np.asarray(bv0)).astype(np.float32),
        np.asarray(bq1, np.float32),
        (np.asarray(bs1) + np.asarray(bv1, np.float32).reshape(H, OUT_CH).mean(0))
        .astype(np.float32),
    ]).view(np.uint8)
    wblob = np.concatenate([
        w0kv.reshape(-1).view(np.uint8),
        np.asarray(Wq0, np.float32).astype(BF).reshape(-1).view(np.uint8),
        np.asarray(Ws0, np.float32).astype(BF).reshape(-1).view(np.uint8),
        np.ascontiguousarray(
            np.concatenate([Wk1, Wv1], 1).astype(BF).reshape(4, 128, 2 * D)
            .transpose(1, 0, 2)).reshape(-1).view(np.uint8),
        np.ascontiguousarray(np.asarray(Wq1, np.float32).astype(BF)
                             .reshape(4, 128, D).transpose(1, 0, 2)).reshape(-1).view(np.uint8),
        np.ascontiguousarray(np.asarray(Ws1, np.float32).astype(BF)
                             .reshape(4, 128, OUT_CH).transpose(1, 0, 2)).reshape(-1).view(np.uint8),
        bias,
    ])
    assert wblob.nbytes == WTOT + bias.nbytes
    wpad = NRANKS * g["WSH"] - wblob.nbytes
    wblob = np.concatenate([wblob, np.zeros(wpad, np.uint8)])

    def wrap_idx(arr):  # [NB, NI] int -> [128, NB*NI//16] int16 (16-wrap, 8x replicated)
        a = arr.reshape(NB, NI // 16, 16).transpose(0, 2, 1)   # [NB, 16, NI//16]
        a = np.tile(a, (1, 8, 1))                               # [NB, 128, NI//16]
        return np.ascontiguousarray(a.transpose(1, 0, 2)
                                    .reshape(128, NB * (NI // 16)).astype(np.int16))

    iota = np.tile(np.arange(128).astype(BF)[None], (128, 1)).view(np.uint8).reshape(-1)

    in_maps = []
    for r in range(NRANKS):
        lo, hi = r * PER, (r + 1) * PER
        m = (dst >= lo) & (dst < hi)
        es, ed, npr = src[m], dst[m] - lo, nprime[m]
        blk = ed // 128
        order = np.argsort(blk, kind="stable")
        ed, npr, blk = ed[order], npr[order], blk[order]
        cnt = np.bincount(blk, minlength=NB)
        assert cnt.max() <= NI, f"block overflow: {cnt.max()} > {NI}"
        kvi = np.zeros((NB, NI), np.int64)
        dqi = np.zeros((NB, NI), np.int64)
        drl = np.full((NB, NI), -1.0, np.float32)
        pos = 0
        for b in range(NB):
            nb = cnt[b]
            sl = slice(pos, pos + nb)
            kvi[b, :nb] = npr[sl]
            dqi[b, :nb] = ed[sl]
            drl[b, :nb] = (ed[sl] % 128).astype(np.float32)
            pos += nb
        xo = np.zeros((PERP, D_IN), BF)
        xo[:PER] = x[lo:hi]
        blob = np.empty(TOT, np.uint8)
        blob[OFFS["x_own"]:OFFS["x_own"] + xo.nbytes] = xo.view(np.uint8).reshape(-1)
        blob[OFFS["wsh"]:OFFS["wsh"] + g["WSH"]] = wblob[r * g["WSH"]:(r + 1) * g["WSH"]]
        kb = wrap_idx(kvi).view(np.uint8).reshape(-1)
        blob[OFFS["kvidx"]:OFFS["kvidx"] + kb.nbytes] = kb
        qb = wrap_idx(dqi).view(np.uint8).reshape(-1)
        blob[OFFS["dqidx"]:OFFS["dqidx"] + qb.nbytes] = qb
        db = np.ascontiguousarray(drl.reshape(NB, T, 128).transpose(2, 0, 1)
                                  .reshape(128, NB * T).astype(BF)).view(np.uint8).reshape(-1)
        blob[OFFS["dstrel"]:OFFS["dstrel"] + db.nbytes] = db
        blob[OFFS["iota"]:OFFS["iota"] + iota.nbytes] = iota
        in_maps.append(dict(blob=blob))
    return in_maps


# ----------------------------------------------------------------------------
# Entry point
# ----------------------------------------------------------------------------

_CACHE = {}
_CACHE_VER = "gnn_v4"
_CACHE_PATH = "/root/.cache/" + _CACHE_VER + ".pkl"


class _NcShim:
    """Minimal stand-in for the compiled Bacc object: only the attributes the
    bass2jax/axon execution path reads."""
    dbg_addr = None
    partition_id_tensor = None
    debug = False
    dbg_callbacks = ()
    target_bir_lowering = False

    def is_finalized(self):
        return True

    def __init__(self, bir, arch, allocs, has_collectives, pid_name=None):
        import types
        self._bir = bir
        self.has_collectives = has_collectives
        self.partition_id_tensor = (types.SimpleNamespace(name=pid_name)
                                    if pid_name else None)
        self.m = types.SimpleNamespace(
            arch=arch, functions=[types.SimpleNamespace(allocations=allocs)])

    def to_json_bytes(self):
        return self._bir


def _save_cache(nc):
    import pickle, os, zlib
    try:
        import concourse.mybir as mybir
        allocs = [a for a in nc.m.functions[0].allocations
                  if isinstance(a, mybir.MemoryLocationSet)]
        payload = dict(bir=zlib.compress(nc.to_json_bytes(), 1),
                       arch=nc.m.arch, allocs=allocs,
                       has_collectives=nc.has_collectives,
                       pid_name=(nc.partition_id_tensor.name
                                 if nc.partition_id_tensor else None))
        os.makedirs(os.path.dirname(_CACHE_PATH), exist_ok=True)
        with open(_CACHE_PATH + ".tmp", "wb") as f:
            pickle.dump(payload, f, protocol=pickle.HIGHEST_PROTOCOL)
        os.replace(_CACHE_PATH + ".tmp", _CACHE_PATH)
    except Exception:
        pass


def _load_cache():
    import pickle, zlib
    try:
        with open(_CACHE_PATH, "rb") as f:
            p = pickle.load(f)
        return _NcShim(zlib.decompress(p["bir"]), p["arch"], p["allocs"],
                       p["has_collectives"], p.get("pid_name"))
    except Exception:
        return None


def _get_program():
    if "nc" not in _CACHE:
        nc = _load_cache()
        if nc is None:
            nc = build_program(FULL_CFG)
            _save_cache(nc)
        _CACHE["nc"] = nc
    return _CACHE["nc"]


def run_on_hw(nc, in_maps, cfg, trace=False):
    from concourse import bass_utils
    g = derive(cfg)
    res = bass_utils.run_bass_kernel_spmd(
        nc, in_maps, core_ids=list(range(g["RANKS"])), trace=trace)
    outs = [np.asarray(res.results[r]["out"][:g["PER"]], np.float32)
            for r in range(g["RANKS"])]
    return np.concatenate(outs, 0), res


def kernel(x, edge_index,
           Wq0, bq0, Wk0, bk0, Wv0, bv0, Ws0, bs0,
           Wq1, bq1, Wk1, bk1, Wv1, bv1, Ws1, bs1):
    nc = _get_program()
    in_maps = host_prep(FULL_CFG, x, edge_index,
                        Wq0, bq0, Wk0, bk0, Wv0, bv0, Ws0, bs0,
                        Wq1, bq1, Wk1, bk1, Wv1, bv1, Ws1, bs1)
    out, _ = run_on_hw(nc, in_maps, FULL_CFG)
    return out
